# revision 20
# baseline (speedup 1.0000x reference)
"""Trainium2 Bass kernel for nn_BoundaryKDV4 (boundary-KL distillation loss).

Contract: kernel(**inputs) takes FULL inputs (preds_S, preds_T, outputs_T:
[2,14,96,96,96] f32), shards across 8 NeuronCores internally, and returns the
FULL output (scalar f32 loss), matching reference semantics.

Sharding: core = (b, hq) with b in {0,1}, hq in {0..3}; each core handles 24
H-slices of one batch. All device inputs are bf16 (host-cast) to halve HBM
traffic. outputs_T shards carry a 1-slice halo on each side and an extra
"mask channel" (index 14) set to -1e38 on valid slices / +1e38 on
out-of-range halo slices, so the argmax one-hot of padding slices is
identically zero with a core-uniform SPMD program.

Fused pipeline over 6 h-regions (4 own H-slices each). Per region g:
  KL: [128, 288, C] chunk; exp via ACT, class sums via single
    tensor_reduce(X) ops (c innermost), d=sT-sS on GpSimd, q=expT*d on DVE;
    pk = sumq/sumT - ln sumT + ln sumS -> DRAM scratch -> [96W,(h,d)] reload.
  Boundary: oT 15-ch tree-max (GpSimd) + broadcast is_ge one-hot (DVE) into a
    d-padded [96, 26, 14, 100] oh tile; H-box = 2 shifted adds (DVE);
    W-box = tridiagonal band matmul, D-box = 3 PSUM-accumulated shifted
    matmuls, batched per own-slice into a [96,3,512] PSUM tile (d%32-chunked
    columns); u=(box-13.5)^2 via one Square ACT per own slice;
    ind = u<169 (DVE tensor_scalar, 4x); np = (u<169)*pk fused via
    scalar_tensor_tensor; n/num accumulated on PE with ones-column matmuls
    into two persistent PSUM banks, columns folded as (d%32, c).
Host combines the 8 partial (n, num) pairs and applies the final
KLDivLoss(mean)-style normalization.
"""

import numpy as np

B, C, H, W, D = 2, 14, 96, 96, 96
CM = C + 1          # channels incl. mask
HQ = H // 4         # 24 h-slices per core
S = HQ + 2          # 26 oT slices incl. halo
WD = W * D          # 9216
V = HQ * WD         # 221184 voxels per core
NCORES = 8
BIG = 1.0e38        # +/- mask channel values (finite in bf16, beats any data)

NG = 6              # h-regions (fused pipeline iterations)
GH = 4              # own h-slices per region
AJ = 288            # free dim per partition per chunk (4*96*96 / 128)
VC = 128 * AJ       # voxels per chunk = 36864

# engine-assignment knobs (NOTE: plain tensor_tensor is NOT legal on the
# Pool/GpSimd engine in this neuronxcc codegen -- keep these False)
D_ON_GPSIMD = False     # d = sT - sS on GpSimd instead of DVE
MAX_ON_GPSIMD = False   # oT channel tree-max on GpSimd instead of DVE

_CACHE = {}


def _patched_act_root():
    """Build an act_info.json that keeps ONLY natural_log_exp_and_others
    (exp+ln+square in one table set) so walrus never thrashes ACT tables
    between the per-iteration Exp/Ln/Square calls."""
    import json
    import os
    import tempfile

    from neuronxcc.driver.Job import Job
    from neuronxcc.driver.jobs.support.FindActInfo import findActInfoFile

    src = findActInfoFile(Job.getPackageDir(), "gen3")
    src_dir = os.path.dirname(src)
    dst_dir = os.path.join(
        tempfile.gettempdir(), "bass_act_root_lnexp_%d" % os.getuid()
    )
    os.makedirs(dst_dir, exist_ok=True)
    for fn in os.listdir(src_dir):
        link = os.path.join(dst_dir, fn)
        if not os.path.exists(link):
            try:
                os.symlink(os.path.join(src_dir, fn), link)
            except OSError:
                pass
    with open(src) as f:
        info = json.load(f)
    info["act_func_sets"] = [
        s
        for s in info["act_func_sets"]
        if s.get("name") == "natural_log_exp_and_others"
    ]
    patched = os.path.join(dst_dir, "act_info_lnexp.json")
    with open(patched, "w") as f:
        json.dump(info, f)
    return patched


def _build_program():
    import functools
    import json
    import os

    patched = _patched_act_root()
    os.environ["BASS_ACT_ROOT_JSON_PATH"] = patched

    # Point bass's table-set chooser at the same pruned act_info walrus
    # compiles with (exp+ln+square all live in natural_log_exp_and_others,
    # so every ACTIVATE resolves to one table set -> no ACT_TABLE_LOAD
    # thrash). bass has no env hook for this, so rebind the reader.
    import concourse.hw_specs as hw_specs
    import concourse.bacc as bacc
    import concourse.mybir as _mb

    @functools.cache
    def _tables(_arch):
        with open(patched) as f:
            info = json.load(f)
        return {
            ent["name"]: {
                _mb.ActivationFunctionType.from_pwp(v)
                for v in ent["act"].keys()
            }
            for ent in info["act_func_sets"]
        }

    hw_specs.get_activation_tables = _tables
    bacc.get_activation_tables = _tables

    # The kernel issues 360 matmuls but only 2 distinct stationaries (the
    # tridiagonal W-band and the ones column); walrus's LDWEIGHTS dedup pass
    # removes the redundant reloads but bass disables it by default.
    import concourse.bass_utils as _bu

    if not getattr(_bu, "_ldw_opt_patched", False):
        _orig_gwa = _bu.get_walrus_args

        def _gwa(*a, **kw):
            args = _orig_gwa(*a, **kw)
            return [
                x.replace("--enable-ldw-opt=false", "--enable-ldw-opt=true")
                if isinstance(x, str)
                else x
                for x in args
            ]

        _bu.get_walrus_args = _gwa
        _bu._ldw_opt_patched = True
    import concourse.mybir as mybir
    from concourse.mybir import AluOpType as alu
    from concourse.mybir import ActivationFunctionType as actf
    from concourse.tile import TileContext
    from contextlib import ExitStack
    import ml_dtypes

    f32 = mybir.dt.float32
    bf16 = mybir.dt.bfloat16
    bfnp = ml_dtypes.bfloat16

    nc = bacc.Bacc("TRN2", target_bir_lowering=False)

    # host-prepared, all bf16:
    #   ot16: (s, w, c, d) with mask channel c=14
    #   ps16/pt16: (g, p, j, c) with voxel v' = p*AJ + j inside region g,
    #     v' = (h_local*96 + w)*96 + d for the 4 own slices of the region
    ot = nc.dram_tensor("ot16", [S, W, CM, D], bf16, kind="ExternalInput")
    ps = nc.dram_tensor("ps16", [NG, 128, C, AJ], bf16, kind="ExternalInput")
    pt = nc.dram_tensor("pt16", [NG, 128, C, AJ], bf16, kind="ExternalInput")
    nn_out = nc.dram_tensor("nn_out", [2, C], f32, kind="ExternalOutput")

    band_np = np.zeros((W, W), dtype=bfnp)
    for i in range(W):
        for j in range(max(0, i - 1), min(W, i + 2)):
            band_np[i, j] = 1.0
    band_h = nc.inline_tensor(band_np, name="bandw")
    ones_h = nc.inline_tensor(np.ones((W, 1), dtype=bfnp), name="onesw")

    with TileContext(nc) as tc, ExitStack() as es:
        # ---------------- constants ----------------
        cpool = es.enter_context(tc.tile_pool(name="consts", bufs=1))
        band_t = cpool.tile([W, W], bf16, name="band_t")
        ones_t = cpool.tile([W, 1], bf16, name="ones_t")
        bias_t = cpool.tile([W, 1], f32, name="bias_t")
        nc.sync.dma_start(band_t[:], band_h[:])
        nc.sync.dma_start(ones_t[:], ones_h[:])
        nc.vector.memset(bias_t[:], -13.5)

        dram_pool = es.enter_context(tc.tile_pool(name="dramp", bufs=1, space="DRAM"))
        pk_dram = dram_pool.tile([NG, 128, AJ], bf16, name="pk_dram")

        # ---------------- persistent SBUF ----------------
        bpool = es.enter_context(tc.tile_pool(name="bconst", bufs=1))
        # one-hot, d-padded to 100 (data at 2..97)
        oh = bpool.tile([W, S, C, 100], bf16, name="oh")
        pk_w = bpool.tile([W, HQ, D], bf16, name="pk_w")
        nc.vector.memset(oh[:, :, :, 0:2], 0.0)
        nc.vector.memset(oh[:, :, :, 98:100], 0.0)

        # ---------------- PSUM ----------------
        psum_acc = es.enter_context(tc.tile_pool(name="psacc", bufs=1, space="PSUM"))
        nacc = psum_acc.tile([1, 448], f32, name="nacc")
        numacc = psum_acc.tile([1, 448], f32, name="numacc")

        with tc.tile_pool(name="aload", bufs=2) as alp, \
             tc.tile_pool(name="awork", bufs=2) as awp, \
             tc.tile_pool(name="asum", bufs=2) as asp, \
             tc.tile_pool(name="avox", bufs=1) as avp, \
             tc.tile_pool(name="otload", bufs=2) as otp, \
             tc.tile_pool(name="treework", bufs=1) as twp, \
             tc.tile_pool(name="hbwork", bufs=1) as hbp, \
             tc.tile_pool(name="gwork", bufs=1) as gp, \
             tc.tile_pool(name="boxps", bufs=2, space="PSUM") as bxp:

            ge_d = nc.gpsimd if D_ON_GPSIMD else nc.vector
            ge_m = nc.gpsimd if MAX_ON_GPSIMD else nc.vector
            first_cs = [True, True]  # per type (n, num): first colsum matmul?

            def emit_tree_cmp(pair):
                """load oT slices 2p, 2p+1; 15-ch tree-max; one-hot cmp"""
                s0 = 2 * pair
                oT_t = otp.tile([W, 2, CM, D], bf16, name="oT_t", tag="oT")
                q = nc.sync if (pair % 2 == 0) else nc.scalar
                q.dma_start(
                    oT_t[:], ot[s0 : s0 + 2].rearrange("s w c d -> w s c d")
                )
                x = oT_t
                w7 = twp.tile([W, 2, 7, D], bf16, name="mw7", tag="mw7")
                w3 = twp.tile([W, 2, 3, D], bf16, name="mw3", tag="mw3")
                m_t = twp.tile([W, 2, D], bf16, name="m_t", tag="m")
                ge_m.tensor_tensor(w7[:], x[:, :, 0:7, :], x[:, :, 7:14, :], alu.max)
                ge_m.tensor_tensor(w3[:], w7[:, :, 0:3, :], w7[:, :, 3:6, :], alu.max)
                ge_m.tensor_tensor(m_t[:], w3[:, :, 0, :], w3[:, :, 1, :], alu.max)
                ge_m.tensor_tensor(m_t[:], m_t[:], w3[:, :, 2, :], alu.max)
                ge_m.tensor_tensor(m_t[:], m_t[:], w7[:, :, 6, :], alu.max)
                ge_m.tensor_tensor(m_t[:], m_t[:], x[:, :, 14, :], alu.max)
                nc.vector.tensor_tensor(
                    oh[:, s0 : s0 + 2, :, 2 : 2 + D],
                    x[:, :, 0:14, :],
                    m_t[:, :, None, :].broadcast_to([W, 2, C, D]),
                    alu.is_ge,
                )

            next_pair = 0
            for g in range(NG):
                # ---- one-hot for oh slices <= 4g+5 (pairs <= 2g+2) ----
                while next_pair <= 2 * g + 2:
                    emit_tree_cmp(next_pair)
                    next_pair += 1

                # ---- KL chunk g ----
                sS_t = alp.tile([128, C, AJ], bf16, name="sS_t", tag="sS")
                sT_t = alp.tile([128, C, AJ], bf16, name="sT_t", tag="sT")
                nc.sync.dma_start(sS_t[:], ps[g])
                nc.scalar.dma_start(sT_t[:], pt[g])

                expT = awp.tile([128, C, AJ], bf16, name="expT", tag="expT")
                expS = awp.tile([128, C, AJ], bf16, name="expS", tag="expS")
                nc.scalar.activation(expT[:], sT_t[:], actf.Exp)
                nc.scalar.activation(expS[:], sS_t[:], actf.Exp)

                sumT_c = asp.tile([128, AJ], bf16, name="sumT_c", tag="sumT")
                sumS_c = asp.tile([128, AJ], bf16, name="sumS_c", tag="sumS")
                sumq_c = asp.tile([128, AJ], bf16, name="sumq_c", tag="sumq")

                def tree_sum(sm, src):
                    # 14-way class sum via 2x-mode TT adds (reduce-X runs 1x)
                    w7s = awp.tile([128, 7, AJ], bf16, name="w7s", tag="w7s")
                    w3s = awp.tile([128, 3, AJ], bf16, name="w3s", tag="w3s")
                    nc.vector.tensor_tensor(
                        w7s[:], src[:, 0:7, :], src[:, 7:14, :], alu.add
                    )
                    nc.vector.tensor_tensor(
                        w3s[:], w7s[:, 0:3, :], w7s[:, 3:6, :], alu.add
                    )
                    nc.vector.tensor_tensor(
                        sm[:], w3s[:, 0, :], w3s[:, 1, :], alu.add
                    )
                    nc.vector.tensor_tensor(sm[:], sm[:], w3s[:, 2, :], alu.add)
                    nc.vector.tensor_tensor(sm[:], sm[:], w7s[:, 6, :], alu.add)

                tree_sum(sumT_c, expT)
                tree_sum(sumS_c, expS)
                # d = sT - sS (in place into sT_t); q = expT*d (into sS_t)
                ge_d.tensor_tensor(sT_t[:], sT_t[:], sS_t[:], alu.subtract)
                nc.vector.tensor_tensor(sS_t[:], expT[:], sT_t[:], alu.mult)
                tree_sum(sumq_c, sS_t)

                # ---- voxel stage g: pk = sumq/sumT - ln sumT + ln sumS ----
                lnT = avp.tile([128, AJ], bf16, name="lnT", tag="lnT")
                lnS = avp.tile([128, AJ], bf16, name="lnS", tag="lnS")
                inv = avp.tile([128, AJ], bf16, name="inv", tag="inv")
                pk_c = avp.tile([128, AJ], bf16, name="pk_c", tag="pk_c")
                nc.scalar.activation(lnT[:], sumT_c[:], actf.Ln)
                nc.scalar.activation(lnS[:], sumS_c[:], actf.Ln)
                nc.scalar.activation(inv[:], lnT[:], actf.Exp, scale=-1.0)
                nc.vector.tensor_tensor(pk_c[:], sumq_c[:], inv[:], alu.mult)
                nc.vector.tensor_tensor(pk_c[:], pk_c[:], lnT[:], alu.subtract)
                nc.vector.tensor_tensor(pk_c[:], pk_c[:], lnS[:], alu.add)
                nc.scalar.dma_start(pk_dram[g], pk_c[:])
                nc.sync.dma_start(
                    pk_w[:, GH * g : GH * (g + 1), :],
                    pk_dram[g]
                    .rearrange("p j -> (p j)")
                    .rearrange("(h w d) -> w h d", h=GH, w=W, d=D),
                )

                # ---- H-box: hb[j] = oh[4g+j] + oh[4g+j+1] + oh[4g+j+2] ----
                hb_t = hbp.tile([W, GH, C, 100], bf16, name="hb_t", tag="hb")
                nc.vector.tensor_tensor(
                    hb_t[:],
                    oh[:, 4 * g : 4 * g + 4, :, :],
                    oh[:, 4 * g + 1 : 4 * g + 5, :, :],
                    alu.add,
                )
                nc.vector.tensor_tensor(
                    hb_t[:], hb_t[:], oh[:, 4 * g + 2 : 4 * g + 6, :, :], alu.add
                )

                # ---- W/D box + indicator + reductions per own slice ----
                # columns ordered (dc, c, dj): dj innermost for 2x-mode TT
                u_t = gp.tile([W, GH, 3, C, 32], bf16, name="u_t", tag="u")
                ind_t = gp.tile([W, GH, 3, C, 32], bf16, name="ind_t", tag="ind")
                np_t = u_t  # np = ind*pk written in place over u (dead after)

                for gi in range(GH):
                    box_ps = bxp.tile([W, 3, 512], f32, name="box_ps", tag="box")
                    for dc in range(3):
                        for dd in range(3):
                            nc.tensor.matmul(
                                box_ps[:, dc, 0:448],
                                band_t[:],
                                hb_t[
                                    :, gi, :, 1 + dd + 32 * dc : 33 + dd + 32 * dc
                                ],
                                start=(dd == 0),
                                stop=(dd == 2),
                            )
                    # u = (box - 13.5)^2 ; boundary iff u < 169 (0<box<27)
                    nc.scalar.activation(
                        u_t[:, gi].rearrange("w dc c dj -> w dc (c dj)"),
                        box_ps[:, :, 0:448],
                        actf.Square,
                        bias=bias_t[:],
                    )
                nc.vector.tensor_scalar(
                    ind_t[:].rearrange("w g dc c dj -> w (g dc c dj)"),
                    u_t[:].rearrange("w g dc c dj -> w (g dc c dj)"),
                    169.0,
                    None,
                    alu.is_lt,
                )
                for gi in range(GH):
                    pk_bc = (
                        pk_w[:, 4 * g + gi, :]
                        .rearrange("w (dc dj) -> w dc dj", dc=3)[:, :, None, :]
                        .broadcast_to([W, 3, C, 32])
                    )
                    nc.vector.tensor_tensor(
                        np_t[:, gi], ind_t[:, gi], pk_bc, alu.mult
                    )

                # ---- colsum matmuls into (d%32, c) accumulators ----
                for gi in range(GH):
                    for ti, (src, accp) in enumerate(
                        ((ind_t, nacc), (np_t, numacc))
                    ):
                        for dc in range(3):
                            rhs = src[:, gi, dc].rearrange("w c dj -> w dj c")
                            is_first = first_cs[ti]
                            first_cs[ti] = False
                            is_last = g == NG - 1 and gi == GH - 1 and dc == 2
                            nc.tensor.matmul(
                                accp[:],
                                ones_t[:],
                                rhs,
                                start=is_first,
                                stop=is_last,
                                skip_group_check=True,
                            )

            # final: reduce (d%32) out of the accumulators, write [2, C]
            res_t = twp.tile([1, 2, C], f32, name="res_t", tag="res")
            nc.vector.tensor_reduce(
                res_t[:, 0, :],
                nacc[:].rearrange("p (dj c) -> p c dj", c=C),
                mybir.AxisListType.X,
                alu.add,
            )
            nc.vector.tensor_reduce(
                res_t[:, 1, :],
                numacc[:].rearrange("p (dj c) -> p c dj", c=C),
                mybir.AxisListType.X,
                alu.add,
            )
            nc.sync.dma_start(
                nn_out[:].rearrange("a c -> (a c)")[None, :],
                res_t[:].rearrange("p a c -> p (a c)"),
            )

    nc.compile()
    return nc


def _get_program():
    if "nc" not in _CACHE:
        _CACHE["nc"] = _build_program()
    return _CACHE["nc"]


def _make_in_maps(preds_S, preds_T, outputs_T):
    import ml_dtypes

    bf = ml_dtypes.bfloat16
    in_maps = []
    for core in range(NCORES):
        b, hq = divmod(core, 4)
        h0 = HQ * hq
        ot15 = np.empty((CM, S, W, D), dtype=np.float32)
        lo, hi = h0 - 1, h0 + HQ + 1
        slo, shi = max(0, lo), min(H, hi)
        ot15[:C, slo - lo : shi - lo] = outputs_T[b, :, slo:shi]
        ot15[C, :] = -BIG
        if lo < 0:
            ot15[:C, 0] = 0.0
            ot15[C, 0] = BIG
        if hi > H:
            ot15[:C, S - 1] = 0.0
            ot15[C, S - 1] = BIG

        def chunked(x):
            # (C, HQ, W, D) -> (NG, 128, C, AJ) bf16
            y = x.reshape(C, NG, 128, AJ).transpose(1, 2, 0, 3)
            return np.ascontiguousarray(y.astype(bf))

        in_maps.append(
            {
                # (c, s, w, d) -> (s, w, c, d): one contiguous run/partition
                "ot16": np.ascontiguousarray(
                    ot15.transpose(1, 2, 0, 3).astype(bf)
                ),
                "ps16": chunked(preds_S[b, :, h0 : h0 + HQ]),
                "pt16": chunked(preds_T[b, :, h0 : h0 + HQ]),
            }
        )
    return in_maps


def _combine(results):
    n = np.zeros((B, C), dtype=np.float64)
    num = np.zeros((B, C), dtype=np.float64)
    for core, res in enumerate(results):
        b = core // 4
        nn = np.asarray(res["nn_out"], dtype=np.float64)
        n[b] += nn[0]
        num[b] += nn[1]
    term = np.where(n > 0, num / (C * np.maximum(n, 1.0)), 0.0)
    return np.float32(term.sum())


def kernel(preds_S, preds_T, outputs_T):
    from concourse.bass_utils import run_bass_kernel_spmd

    nc = _get_program()
    in_maps = _make_in_maps(preds_S, preds_T, outputs_T)
    res = run_bass_kernel_spmd(nc, in_maps, core_ids=list(range(NCORES)))
    return np.asarray(_combine(res.results))
